# revision 30
# baseline (speedup 1.0000x reference)
"""GNN mean-aggregation message passing on 8 Trainium2 NeuronCores.

out[t] = mean_{e: tgt[e]==t} x[src[e]]   (0 if no incoming edges)

Strategy (target-sharded, uniform SPMD schedule):
  - Each core owns a contiguous range of 12544 targets (98 groups of 128).
  - Host packs x as bf16 hi|lo pairs -> [N_pad, 128] bf16 (256B rows). The
    hi+lo split recovers ~fp32 precision after the f32 PSUM accumulation.
  - Edges are routed to the owning core, ordered by (supergroup, src-chunk,
    target) and packed into 128-edge slots. Every slot is bound (at compile
    time, uniformly across cores) to a target group g; its edges may only
    reference groups {g, g+1}. Two matmuls per slot (one-hot selection
    matrices vs. iota 0..127 / 128..255) accumulate into per-group PSUM.
  - Sources are gathered straight from HBM with dma_gather (int16 indices,
    4 chunks of 25088 rows to fit the int16 range).
  - Finalize per group: (psum_hi + psum_lo) * inv_count -> DMA out.
"""
import sys

sys.path.insert(0, "/opt/trn_rl_repo")

import numpy as np
import ml_dtypes

bf16 = ml_dtypes.bfloat16

# ---- problem constants (hardcoded per harness contract) ----
N, F, E = 100000, 64, 1600000
P = 128
NCORES = 8
TPC = 12544                  # targets per core (= 98 * 128)
GPC = TPC // P               # 98 groups per core
NCHUNKS = 4
CHUNK = 25088                # source rows per chunk (< 32768 for int16 idx)
NPAD = NCHUNKS * CHUNK       # 100352 padded source rows
GSG = 15                     # groups per supergroup; gs+1=16 slots -> 4 PSUM banks
                             # (two supergroups double-buffer the 8 banks)
SG_SIZES = [GSG] * (GPC // GSG) + ([GPC % GSG] if GPC % GSG else [])  # [12]*8+[2]
ELEM = 2 * F                 # 128 bf16 per packed row = 256B
SENT = 384.0                 # sentinel row id for pad edges (no iota match)
SEL_PIECE = 16               # slots per one-hot compare instruction


def _chunk_order(sched, s):
    return sorted(range(NCHUNKS), key=lambda c: (-len(sched[s][c][0]), c))


def _balance_groups(d):
    """Greedy 4-dim balanced partition of TPC targets into GPC groups of P.

    d: [TPC, NCHUNKS] per-target per-chunk edge counts. Returns (G, R):
    group and row-in-group per target. Balancing per-(group, chunk) sums
    across cores aligns the slot schedule so most slots are pure.
    """
    tot = d.sum(1)
    order = np.argsort(-tot, kind="stable")
    L = np.zeros((GPC, NCHUNKS), np.float64)
    n = np.zeros(GPC, np.int64)
    G = np.empty(TPC, np.int32)
    R = np.empty(TPC, np.int32)
    for t in order:
        cand = L + d[t]
        cost = (cand * cand).sum(1)
        cost[n >= P] = np.inf
        g = int(np.argmin(cost))
        G[t] = g
        R[t] = n[g]
        L[g] += d[t]
        n[g] += 1
    assert (n == P).all()
    return G, R


def _host_prep(x, edge_idx):
    """Build per-core device arrays and the shared slot schedule."""
    x = np.asarray(x, np.float32)
    src = np.asarray(edge_idx[0], np.int64)
    tgt = np.asarray(edge_idx[1], np.int64)

    # packed hi|lo bf16 table
    hi = x.astype(bf16)
    lo = (x - hi.astype(np.float32)).astype(bf16)
    xp = np.zeros((NPAD, ELEM), bf16)
    xp[:N, :F] = hi
    xp[:N, F:] = lo

    inv_cnt = np.bincount(tgt, minlength=NCORES * TPC).astype(np.float32)
    inv_cnt = 1.0 / np.maximum(inv_cnt, 1.0)

    core = tgt // TPC
    chunk = src // CHUNK
    t_loc = tgt - core * TPC

    # per-core balanced target -> (group, row) mapping
    d = np.zeros((NCORES, TPC, NCHUNKS), np.int32)
    np.add.at(d, (core, t_loc, chunk), 1)
    Gm = np.empty((NCORES, TPC), np.int32)
    Rm = np.empty((NCORES, TPC), np.int32)
    for k in range(NCORES):
        Gm[k], Rm[k] = _balance_groups(d[k])

    gl = Gm[core, t_loc].astype(np.int64)   # local group 0..97
    grow = Rm[core, t_loc].astype(np.int64)  # row within group
    sg = np.minimum(gl // GSG, len(SG_SIZES) - 1)
    order = np.lexsort((gl, chunk, sg, core))

    # bin id = ((core * NSG + sg) * NCHUNKS + c); edges of each bin are
    # contiguous in `order` and sorted by tgt.
    nsg = len(SG_SIZES)
    bin_id = (core * nsg + sg) * NCHUNKS + chunk
    bin_sizes = np.bincount(bin_id, minlength=NCORES * nsg * NCHUNKS)
    bin_starts = np.zeros(NCORES * nsg * NCHUNKS + 1, np.int64)
    np.cumsum(bin_sizes, out=bin_starts[1:])

    gl_sorted = gl[order]
    grow_sorted = grow[order]
    src_sorted = src[order]
    chunk_sorted = chunk[order]

    # ---- build shared schedule: per (sg, c) the block label list ----
    sched = []   # sched[sg][c] = (labels, pure) per-slot arrays (bin-local h)
    for s in range(nsg):
        gs = SG_SIZES[s]
        row = []
        for c in range(NCHUNKS):
            # per-core group counts in this bin
            e_kh = np.zeros((NCORES, gs), np.int64)
            for k in range(NCORES):
                b = (k * nsg + s) * NCHUNKS + c
                seg = gl_sorted[bin_starts[b]:bin_starts[b + 1]] - s * GSG
                if seg.size:
                    e_kh[k] = np.bincount(seg, minlength=gs)
            labels = []
            r = e_kh[:, 0].astype(np.int64)
            for h in range(gs):
                s_h = int(np.ceil(r / P).max())
                labels.extend([h] * s_h)
                cap = s_h * P - r
                if h + 1 < gs:
                    r = np.maximum(0, e_kh[:, h + 1] - cap)
                else:
                    assert (cap >= 0).all()
            labels = np.asarray(labels, np.int64)
            row.append((labels, np.ones(len(labels), bool)))
        sched.append(row)

    tot_slots = sum(len(row[c][0]) for row in sched for c in range(NCHUNKS))
    tot = tot_slots * P

    # ---- per-core edge placement into the uniform slot stream ----
    # A slot is "pure" iff for EVERY core all its edges belong to the slot's
    # own group h (no spill into h+1): pure slots need only a 128-wide
    # one-hot and a single matmul.
    src_local = np.zeros((NCORES, tot), np.int16)
    trow = np.full((NCORES, tot), SENT, np.float32)
    for k in range(NCORES):
        base = 0
        for s in range(nsg):
            for c in _chunk_order(sched, s):
                labels, pure = sched[s][c]
                b = (k * nsg + s) * NCHUNKS + c
                lo_i, hi_i = bin_starts[b], bin_starts[b + 1]
                garr = gl_sorted[lo_i:hi_i] - s * GSG
                p = 0
                for bi, h in enumerate(labels):
                    upper = np.searchsorted(garr, h + 1, side="right")
                    take = min(P, upper - p)
                    if take > 0:
                        sl = slice(lo_i + p, lo_i + p + take)
                        pos = base + bi * P
                        src_local[k, pos:pos + take] = (
                            src_sorted[sl] - chunk_sorted[sl] * CHUNK
                        ).astype(np.int16)
                        spill = garr[p:p + take] - h
                        trow[k, pos:pos + take] = (
                            grow_sorted[sl] + P * spill
                        ).astype(np.float32)
                        if spill.any():
                            pure[bi] = False
                        p += take
                assert p == hi_i - lo_i, (
                    f"core {k} sg {s} c {c}: placed {p} of {hi_i - lo_i}"
                )
                base += len(labels) * P
        assert base == tot

    # ---- reorder slots within each bin: pure first, then mixed ----
    base = 0
    for s in range(nsg):
        for c in _chunk_order(sched, s):
            labels, pure = sched[s][c]
            w = len(labels)
            perm = np.argsort(~pure, kind="stable")
            if not np.array_equal(perm, np.arange(w)):
                sched[s][c] = (labels[perm], pure[perm])
                blk = slice(base, base + w * P)
                for arr in (src_local, trow):
                    v = arr[:, blk].reshape(NCORES, w, P)
                    arr[:, blk] = v[:, perm, :].reshape(NCORES, w * P)
            base += w * P
    assert base == tot

    # device layouts
    idx_dev = [
        np.tile(src_local[k].reshape(tot // 16, 16).T, (8, 1)).copy()
        for k in range(NCORES)
    ]
    trow_dev = [
        src_arr.reshape(tot // P, P).T.astype(bf16).copy()
        for src_arr in trow
    ]
    # invc in the permuted (group, row) layout; unperm maps device output
    # row g*P+r back to the original local target id.
    invc_dev = []
    unperm = []
    for k in range(NCORES):
        arr = np.empty((GPC, P), np.float32)
        arr[Gm[k], Rm[k]] = inv_cnt[k * TPC:(k + 1) * TPC]
        invc_dev.append(arr.T.copy())
        unperm.append((Gm[k].astype(np.int64) * P + Rm[k]).copy())
    return xp, idx_dev, trow_dev, invc_dev, sched, tot, unperm


def _build_program(sched, tot):
    from concourse import bacc, mybir, tile

    nsg = len(SG_SIZES)
    max_w = max(len(sched[s][c][0]) for s in range(nsg) for c in range(NCHUNKS))

    nc = bacc.Bacc(None, target_bir_lowering=False, debug=False,
                   num_swdge_queues=4)
    t_x = nc.dram_tensor("xp", [NPAD, ELEM], mybir.dt.bfloat16, kind="ExternalInput")
    t_idx = nc.dram_tensor("idx", [P, tot // 16], mybir.dt.int16, kind="ExternalInput")
    t_trow = nc.dram_tensor("trow", [P, tot // P], mybir.dt.bfloat16, kind="ExternalInput")
    t_invc = nc.dram_tensor("invc", [P, GPC], mybir.dt.float32, kind="ExternalInput")
    t_out = nc.dram_tensor("out", [TPC, F], mybir.dt.float32, kind="ExternalOutput")
    out_view = t_out[:].rearrange("(g r) f -> r g f", r=P)

    with tile.TileContext(nc) as tc:
        with (
            tc.tile_pool(name="const", bufs=1) as cpool,
            tc.tile_pool(name="msgs", bufs=8) as mpool,
            tc.tile_pool(name="selp", bufs=4) as sppool,
            tc.tile_pool(name="selm", bufs=4) as smpool,
            tc.tile_pool(name="stage", bufs=2) as stpool,
            tc.tile_pool(name="psum", bufs=8, space="PSUM") as ppool,
        ):
            idx_t = cpool.tile([P, tot // 16], mybir.dt.int16)
            trow_t = cpool.tile([P, tot // P], mybir.dt.bfloat16)
            invc_t = cpool.tile([P, GPC], mybir.dt.float32)
            iota_i = cpool.tile([P, 2 * P], mybir.dt.int32)
            iota_b = cpool.tile([P, 2 * P], mybir.dt.bfloat16)
            zero_t = cpool.tile([P, 4 * P], mybir.dt.float32)

            # split meta loads per supergroup so the first gather starts early
            sg_w = [sum(len(sched[s][c][0]) for c in range(NCHUNKS)) for s in range(nsg)]
            off = 0
            for s in range(nsg):
                w = sg_w[s]
                if w == 0:
                    continue
                nc.sync.dma_start(out=idx_t[:, off * 8:(off + w) * 8],
                                  in_=t_idx[:, off * 8:(off + w) * 8])
                nc.sync.dma_start(out=trow_t[:, off:off + w],
                                  in_=t_trow[:, off:off + w])
                off += w
            nc.sync.dma_start(out=invc_t[:], in_=t_invc[:])
            nc.gpsimd.iota(iota_i[:], pattern=[[1, 2 * P]], base=0, channel_multiplier=0)
            nc.vector.tensor_copy(out=iota_b[:], in_=iota_i[:])
            nc.vector.memset(zero_t[:], 0.0)

            slot_off = 0     # global slot offset in the stream
            g_base = 0       # global group offset
            qload = [0, 0, 0, 0]   # per-queue gathered-idx load (greedy LB)
            for s in range(nsg):
                gs = SG_SIZES[s]
                nslots_psum = gs + 1
                nbanks = (nslots_psum + 3) // 4
                pts = [
                    ppool.tile([P, 4 * P], mybir.dt.float32, name=f"ps{s}_{b}", tag="ps")
                    for b in range(nbanks)
                ]
                touched_banks = set()

                def pslot(j):
                    return pts[j // 4][:, (j % 4) * P:(j % 4 + 1) * P]

                def mm(j, lhsT, rhs):
                    # start=True on the first matmul into each PSUM bank
                    # resets the whole bank, replacing explicit zero-fills
                    first = (j // 4) not in touched_banks
                    touched_banks.add(j // 4)
                    nc.tensor.matmul(
                        pslot(j), lhsT=lhsT, rhs=rhs,
                        start=first, stop=False, skip_group_check=True,
                    )

                # shorter calls in the (small) last supergroup shrink the
                # end-of-kernel drain tail
                call_w = 48 if s < nsg - 1 else 24
                for c in _chunk_order(sched, s):
                    labels, pure = sched[s][c]
                    w = len(labels)
                    if w == 0:
                        continue
                    # one msgs tile + gather call per <=48-slot range (ring
                    # wraps stall above ~8K idx); round-robin the 4 SWDGE
                    # queues (each queue runs on its own Q7 cpu pair) and
                    # keep calls independent so 4 desc-gens overlap from the
                    # very first round.
                    nparts = -(-w // call_w)
                    bounds = [w * i // nparts for i in range(nparts + 1)]
                    for g0, g1 in zip(bounds[:-1], bounds[1:]):
                        gw = g1 - g0
                        msgs_t = mpool.tile([P, 48, ELEM], mybir.dt.bfloat16, name="msgs")
                        q = min(range(4), key=lambda i: qload[i])
                        qload[q] += gw
                        nc.gpsimd.dma_gather(
                            out_ap=msgs_t[:, 0:gw, :],
                            in_ap=t_x[c * CHUNK:(c + 1) * CHUNK, :],
                            idxs_ap=idx_t[:, (slot_off + g0) * 8:(slot_off + g0 + gw) * 8],
                            num_idxs=gw * P,
                            num_idxs_reg=gw * P,
                            elem_size=ELEM,
                            single_packet=False,
                            queue_num=q,
                        )
                        # pieces of consecutive same-kind slots (pure slots
                        # were sorted first within the bin): pure pieces
                        # compare only 128 one-hot columns and emit a single
                        # matmul per slot.
                        p0 = g0
                        while p0 < g0 + gw:
                            is_pure = bool(pure[p0])
                            p1 = p0
                            while (p1 < g0 + gw and p1 - p0 < SEL_PIECE
                                   and bool(pure[p1]) == is_pure):
                                p1 += 1
                            pw = p1 - p0
                            cols = P if is_pure else 2 * P
                            pool_ = sppool if is_pure else smpool
                            sel_t = pool_.tile(
                                [P, SEL_PIECE, cols], mybir.dt.bfloat16,
                                name="selp" if is_pure else "selm",
                            )
                            nc.vector.tensor_tensor(
                                out=sel_t[:, :pw, :],
                                in0=trow_t[:, slot_off + p0:slot_off + p0 + pw]
                                .to_broadcast([P, pw, cols]),
                                in1=iota_b[:, None, 0:cols].to_broadcast([P, pw, cols]),
                                op=mybir.AluOpType.is_equal,
                            )
                            for si in range(pw):
                                h = int(labels[p0 + si])
                                mm(h, sel_t[:, si, 0:P], msgs_t[:, p0 + si - g0, :])
                                if not is_pure:
                                    mm(h + 1, sel_t[:, si, P:2 * P],
                                       msgs_t[:, p0 + si - g0, :])
                            p0 = p1
                    slot_off += w

                for b in range(nbanks):
                    if b not in touched_banks:
                        nc.vector.tensor_copy(out=pts[b][:], in_=zero_t[:])
                stage_t = stpool.tile([P, GSG, F], mybir.dt.float32, name="stage")
                for j in range(gs):
                    tmp_t = stpool.tile([P, F], mybir.dt.float32, name="tmp", tag="tmp")
                    pre_t = stpool.tile([P, F], mybir.dt.float32, name="pre", tag="pre")
                    nc.scalar.copy(out=tmp_t[:], in_=pslot(j)[:, 0:F])
                    nc.vector.tensor_add(
                        out=pre_t[:],
                        in0=tmp_t[:],
                        in1=pslot(j)[:, F:2 * F],
                    )
                    nc.scalar.activation(
                        out=stage_t[:, j, :],
                        in_=pre_t[:],
                        func=mybir.ActivationFunctionType.Copy,
                        scale=invc_t[:, g_base + j:g_base + j + 1],
                    )
                nc.sync.dma_start(
                    out=out_view[:, g_base:g_base + gs, :],
                    in_=stage_t[:, :gs, :],
                )
                g_base += gs

    nc.compile()
    return nc


def _run(x, edge_idx, trace=False, tmpdir=None):
    from concourse.bass_utils import run_bass_kernel_spmd

    xp, idx_dev, trow_dev, invc_dev, sched, tot, unperm = _host_prep(x, edge_idx)
    nc = _build_program(sched, tot)
    in_maps = [
        {"xp": xp, "idx": idx_dev[k], "trow": trow_dev[k], "invc": invc_dev[k]}
        for k in range(NCORES)
    ]
    kw = dict(trace=True, tmpdir=tmpdir) if trace else {}
    res = run_bass_kernel_spmd(nc, in_maps, list(range(NCORES)), **kw)
    out = np.concatenate(
        [res.results[k]["out"][unperm[k]] for k in range(NCORES)], axis=0
    )
    return out[:N], res.exec_time_ns


def kernel(x, edge_idx):
    return _run(x, edge_idx)[0]



# revision 32
# speedup vs baseline: 1.0091x; 1.0091x over previous
"""GNN mean-aggregation message passing on 8 Trainium2 NeuronCores.

out[t] = mean_{e: tgt[e]==t} x[src[e]]   (0 if no incoming edges)

Strategy (target-sharded, uniform SPMD schedule):
  - Each core owns a contiguous range of 12544 targets (98 groups of 128).
  - Host packs x as bf16 hi|lo pairs -> [N_pad, 128] bf16 (256B rows). The
    hi+lo split recovers ~fp32 precision after the f32 PSUM accumulation.
  - Edges are routed to the owning core, ordered by (supergroup, src-chunk,
    target) and packed into 128-edge slots. Every slot is bound (at compile
    time, uniformly across cores) to a target group g; its edges may only
    reference groups {g, g+1}. Two matmuls per slot (one-hot selection
    matrices vs. iota 0..127 / 128..255) accumulate into per-group PSUM.
  - Sources are gathered straight from HBM with dma_gather (int16 indices,
    4 chunks of 25088 rows to fit the int16 range).
  - Finalize per group: (psum_hi + psum_lo) * inv_count -> DMA out.
"""
import sys

sys.path.insert(0, "/opt/trn_rl_repo")

import numpy as np
import ml_dtypes

bf16 = ml_dtypes.bfloat16

# ---- problem constants (hardcoded per harness contract) ----
N, F, E = 100000, 64, 1600000
P = 128
NCORES = 8
TPC = 12544                  # targets per core (= 98 * 128)
GPC = TPC // P               # 98 groups per core
NCHUNKS = 4
CHUNK = 25088                # source rows per chunk (< 32768 for int16 idx)
NPAD = NCHUNKS * CHUNK       # 100352 padded source rows
GSG = 22                     # groups per supergroup; gs+1=23 slots -> 6 PSUM banks
SG_SIZES = [GSG] * (GPC // GSG) + ([GPC % GSG] if GPC % GSG else [])  # [12]*8+[2]
ELEM = 2 * F                 # 128 bf16 per packed row = 256B
SENT = 384.0                 # sentinel row id for pad edges (no iota match)
SEL_PIECE = 16               # slots per one-hot compare instruction


def _chunk_order(sched, s):
    return sorted(range(NCHUNKS), key=lambda c: (-len(sched[s][c][0]), c))


def _balance_groups(d):
    """Greedy 4-dim balanced partition of TPC targets into GPC groups of P.

    d: [TPC, NCHUNKS] per-target per-chunk edge counts. Returns (G, R):
    group and row-in-group per target. Balancing per-(group, chunk) sums
    across cores aligns the slot schedule so most slots are pure.
    """
    tot = d.sum(1)
    order = np.argsort(-tot, kind="stable")
    L = np.zeros((GPC, NCHUNKS), np.float64)
    n = np.zeros(GPC, np.int64)
    G = np.empty(TPC, np.int32)
    R = np.empty(TPC, np.int32)
    for t in order:
        cand = L + d[t]
        cost = (cand * cand).sum(1)
        cost[n >= P] = np.inf
        g = int(np.argmin(cost))
        G[t] = g
        R[t] = n[g]
        L[g] += d[t]
        n[g] += 1
    assert (n == P).all()
    return G, R


def _host_prep(x, edge_idx):
    """Build per-core device arrays and the shared slot schedule."""
    x = np.asarray(x, np.float32)
    src = np.asarray(edge_idx[0], np.int64)
    tgt = np.asarray(edge_idx[1], np.int64)

    # packed hi|lo bf16 table
    hi = x.astype(bf16)
    lo = (x - hi.astype(np.float32)).astype(bf16)
    xp = np.zeros((NPAD, ELEM), bf16)
    xp[:N, :F] = hi
    xp[:N, F:] = lo

    inv_cnt = np.bincount(tgt, minlength=NCORES * TPC).astype(np.float32)
    inv_cnt = 1.0 / np.maximum(inv_cnt, 1.0)

    core = tgt // TPC
    chunk = src // CHUNK
    t_loc = tgt - core * TPC

    # per-core balanced target -> (group, row) mapping
    d = np.zeros((NCORES, TPC, NCHUNKS), np.int32)
    np.add.at(d, (core, t_loc, chunk), 1)
    Gm = np.empty((NCORES, TPC), np.int32)
    Rm = np.empty((NCORES, TPC), np.int32)
    for k in range(NCORES):
        Gm[k], Rm[k] = _balance_groups(d[k])

    gl = Gm[core, t_loc].astype(np.int64)   # local group 0..97
    grow = Rm[core, t_loc].astype(np.int64)  # row within group
    sg = np.minimum(gl // GSG, len(SG_SIZES) - 1)
    order = np.lexsort((gl, chunk, sg, core))

    # bin id = ((core * NSG + sg) * NCHUNKS + c); edges of each bin are
    # contiguous in `order` and sorted by tgt.
    nsg = len(SG_SIZES)
    bin_id = (core * nsg + sg) * NCHUNKS + chunk
    bin_sizes = np.bincount(bin_id, minlength=NCORES * nsg * NCHUNKS)
    bin_starts = np.zeros(NCORES * nsg * NCHUNKS + 1, np.int64)
    np.cumsum(bin_sizes, out=bin_starts[1:])

    gl_sorted = gl[order]
    grow_sorted = grow[order]
    src_sorted = src[order]
    chunk_sorted = chunk[order]

    # ---- build shared schedule: per (sg, c) the block label list ----
    sched = []   # sched[sg][c] = (labels, pure) per-slot arrays (bin-local h)
    for s in range(nsg):
        gs = SG_SIZES[s]
        row = []
        for c in range(NCHUNKS):
            # per-core group counts in this bin
            e_kh = np.zeros((NCORES, gs), np.int64)
            for k in range(NCORES):
                b = (k * nsg + s) * NCHUNKS + c
                seg = gl_sorted[bin_starts[b]:bin_starts[b + 1]] - s * GSG
                if seg.size:
                    e_kh[k] = np.bincount(seg, minlength=gs)
            labels = []
            r = e_kh[:, 0].astype(np.int64)
            for h in range(gs):
                s_h = int(np.ceil(r / P).max())
                labels.extend([h] * s_h)
                cap = s_h * P - r
                if h + 1 < gs:
                    r = np.maximum(0, e_kh[:, h + 1] - cap)
                else:
                    assert (cap >= 0).all()
            labels = np.asarray(labels, np.int64)
            row.append((labels, np.ones(len(labels), bool)))
        sched.append(row)

    tot_slots = sum(len(row[c][0]) for row in sched for c in range(NCHUNKS))
    tot = tot_slots * P

    # ---- per-core edge placement into the uniform slot stream ----
    # A slot is "pure" iff for EVERY core all its edges belong to the slot's
    # own group h (no spill into h+1): pure slots need only a 128-wide
    # one-hot and a single matmul.
    src_local = np.zeros((NCORES, tot), np.int16)
    trow = np.full((NCORES, tot), SENT, np.float32)
    for k in range(NCORES):
        base = 0
        for s in range(nsg):
            for c in _chunk_order(sched, s):
                labels, pure = sched[s][c]
                b = (k * nsg + s) * NCHUNKS + c
                lo_i, hi_i = bin_starts[b], bin_starts[b + 1]
                garr = gl_sorted[lo_i:hi_i] - s * GSG
                p = 0
                for bi, h in enumerate(labels):
                    upper = np.searchsorted(garr, h + 1, side="right")
                    take = min(P, upper - p)
                    if take > 0:
                        sl = slice(lo_i + p, lo_i + p + take)
                        pos = base + bi * P
                        src_local[k, pos:pos + take] = (
                            src_sorted[sl] - chunk_sorted[sl] * CHUNK
                        ).astype(np.int16)
                        spill = garr[p:p + take] - h
                        trow[k, pos:pos + take] = (
                            grow_sorted[sl] + P * spill
                        ).astype(np.float32)
                        if spill.any():
                            pure[bi] = False
                        p += take
                assert p == hi_i - lo_i, (
                    f"core {k} sg {s} c {c}: placed {p} of {hi_i - lo_i}"
                )
                base += len(labels) * P
        assert base == tot

    # ---- reorder slots within each bin: pure first, then mixed ----
    base = 0
    for s in range(nsg):
        for c in _chunk_order(sched, s):
            labels, pure = sched[s][c]
            w = len(labels)
            perm = np.argsort(~pure, kind="stable")
            if not np.array_equal(perm, np.arange(w)):
                sched[s][c] = (labels[perm], pure[perm])
                blk = slice(base, base + w * P)
                for arr in (src_local, trow):
                    v = arr[:, blk].reshape(NCORES, w, P)
                    arr[:, blk] = v[:, perm, :].reshape(NCORES, w * P)
            base += w * P
    assert base == tot

    # device layouts
    idx_dev = [
        np.tile(src_local[k].reshape(tot // 16, 16).T, (8, 1)).copy()
        for k in range(NCORES)
    ]
    trow_dev = [
        src_arr.reshape(tot // P, P).T.astype(bf16).copy()
        for src_arr in trow
    ]
    # invc in the permuted (group, row) layout; unperm maps device output
    # row g*P+r back to the original local target id.
    invc_dev = []
    unperm = []
    for k in range(NCORES):
        arr = np.empty((GPC, P), np.float32)
        arr[Gm[k], Rm[k]] = inv_cnt[k * TPC:(k + 1) * TPC]
        invc_dev.append(arr.T.copy())
        unperm.append((Gm[k].astype(np.int64) * P + Rm[k]).copy())
    return xp, idx_dev, trow_dev, invc_dev, sched, tot, unperm


def _build_program(sched, tot):
    from concourse import bacc, mybir, tile

    nsg = len(SG_SIZES)
    max_w = max(len(sched[s][c][0]) for s in range(nsg) for c in range(NCHUNKS))

    nc = bacc.Bacc(None, target_bir_lowering=False, debug=False,
                   num_swdge_queues=4)
    t_x = nc.dram_tensor("xp", [NPAD, ELEM], mybir.dt.bfloat16, kind="ExternalInput")
    t_idx = nc.dram_tensor("idx", [P, tot // 16], mybir.dt.int16, kind="ExternalInput")
    t_trow = nc.dram_tensor("trow", [P, tot // P], mybir.dt.bfloat16, kind="ExternalInput")
    t_invc = nc.dram_tensor("invc", [P, GPC], mybir.dt.float32, kind="ExternalInput")
    t_out = nc.dram_tensor("out", [TPC, F], mybir.dt.float32, kind="ExternalOutput")
    out_view = t_out[:].rearrange("(g r) f -> r g f", r=P)

    with tile.TileContext(nc) as tc:
        with (
            tc.tile_pool(name="const", bufs=1) as cpool,
            tc.tile_pool(name="msgs", bufs=8) as mpool,
            tc.tile_pool(name="selp", bufs=4) as sppool,
            tc.tile_pool(name="selm", bufs=4) as smpool,
            tc.tile_pool(name="stage", bufs=2) as stpool,
            tc.tile_pool(name="psum", bufs=8, space="PSUM") as ppool,
        ):
            idx_t = cpool.tile([P, tot // 16], mybir.dt.int16)
            trow_t = cpool.tile([P, tot // P], mybir.dt.bfloat16)
            invc_t = cpool.tile([P, GPC], mybir.dt.float32)
            iota_i = cpool.tile([P, 2 * P], mybir.dt.int32)
            iota_b = cpool.tile([P, 2 * P], mybir.dt.bfloat16)
            zero_t = cpool.tile([P, 4 * P], mybir.dt.float32)

            # split meta loads per supergroup so the first gather starts early
            sg_w = [sum(len(sched[s][c][0]) for c in range(NCHUNKS)) for s in range(nsg)]
            off = 0
            for s in range(nsg):
                w = sg_w[s]
                if w == 0:
                    continue
                nc.sync.dma_start(out=idx_t[:, off * 8:(off + w) * 8],
                                  in_=t_idx[:, off * 8:(off + w) * 8])
                nc.sync.dma_start(out=trow_t[:, off:off + w],
                                  in_=t_trow[:, off:off + w])
                off += w
            nc.sync.dma_start(out=invc_t[:], in_=t_invc[:])
            nc.gpsimd.iota(iota_i[:], pattern=[[1, 2 * P]], base=0, channel_multiplier=0)
            nc.vector.tensor_copy(out=iota_b[:], in_=iota_i[:])
            nc.vector.memset(zero_t[:], 0.0)

            slot_off = 0     # global slot offset in the stream
            g_base = 0       # global group offset
            qload = [0, 0, 0, 0]   # per-queue gathered-idx load (greedy LB)
            for s in range(nsg):
                gs = SG_SIZES[s]
                nslots_psum = gs + 1
                nbanks = (nslots_psum + 3) // 4
                pts = [
                    ppool.tile([P, 4 * P], mybir.dt.float32, name=f"ps{s}_{b}", tag="ps")
                    for b in range(nbanks)
                ]
                touched_banks = set()

                def pslot(j):
                    return pts[j // 4][:, (j % 4) * P:(j % 4 + 1) * P]

                def mm(j, lhsT, rhs):
                    # start=True on the first matmul into each PSUM bank
                    # resets the whole bank, replacing explicit zero-fills
                    first = (j // 4) not in touched_banks
                    touched_banks.add(j // 4)
                    nc.tensor.matmul(
                        pslot(j), lhsT=lhsT, rhs=rhs,
                        start=first, stop=False, skip_group_check=True,
                    )

                # shorter calls in the (small) last supergroup shrink the
                # end-of-kernel drain tail
                call_w = 32 if s < nsg - 1 else 24
                for c in _chunk_order(sched, s):
                    labels, pure = sched[s][c]
                    w = len(labels)
                    if w == 0:
                        continue
                    # one msgs tile + gather call per <=48-slot range (ring
                    # wraps stall above ~8K idx); round-robin the 4 SWDGE
                    # queues (each queue runs on its own Q7 cpu pair) and
                    # keep calls independent so 4 desc-gens overlap from the
                    # very first round.
                    nparts = -(-w // call_w)
                    bounds = [w * i // nparts for i in range(nparts + 1)]
                    for g0, g1 in zip(bounds[:-1], bounds[1:]):
                        gw = g1 - g0
                        msgs_t = mpool.tile([P, 48, ELEM], mybir.dt.bfloat16, name="msgs")
                        q = min(range(4), key=lambda i: qload[i])
                        qload[q] += gw
                        nc.gpsimd.dma_gather(
                            out_ap=msgs_t[:, 0:gw, :],
                            in_ap=t_x[c * CHUNK:(c + 1) * CHUNK, :],
                            idxs_ap=idx_t[:, (slot_off + g0) * 8:(slot_off + g0 + gw) * 8],
                            num_idxs=gw * P,
                            num_idxs_reg=gw * P,
                            elem_size=ELEM,
                            single_packet=False,
                            queue_num=q,
                        )
                        # pieces of consecutive same-kind slots (pure slots
                        # were sorted first within the bin): pure pieces
                        # compare only 128 one-hot columns and emit a single
                        # matmul per slot.
                        p0 = g0
                        while p0 < g0 + gw:
                            is_pure = bool(pure[p0])
                            p1 = p0
                            while (p1 < g0 + gw and p1 - p0 < SEL_PIECE
                                   and bool(pure[p1]) == is_pure):
                                p1 += 1
                            pw = p1 - p0
                            cols = P if is_pure else 2 * P
                            pool_ = sppool if is_pure else smpool
                            sel_t = pool_.tile(
                                [P, SEL_PIECE, cols], mybir.dt.bfloat16,
                                name="selp" if is_pure else "selm",
                            )
                            nc.vector.tensor_tensor(
                                out=sel_t[:, :pw, :],
                                in0=trow_t[:, slot_off + p0:slot_off + p0 + pw]
                                .to_broadcast([P, pw, cols]),
                                in1=iota_b[:, None, 0:cols].to_broadcast([P, pw, cols]),
                                op=mybir.AluOpType.is_equal,
                            )
                            for si in range(pw):
                                h = int(labels[p0 + si])
                                mm(h, sel_t[:, si, 0:P], msgs_t[:, p0 + si - g0, :])
                                if not is_pure:
                                    mm(h + 1, sel_t[:, si, P:2 * P],
                                       msgs_t[:, p0 + si - g0, :])
                            p0 = p1
                    slot_off += w

                for b in range(nbanks):
                    if b not in touched_banks:
                        nc.vector.tensor_copy(out=pts[b][:], in_=zero_t[:])
                stage_t = stpool.tile([P, GSG, F], mybir.dt.float32, name="stage")
                for j in range(gs):
                    tmp_t = stpool.tile([P, F], mybir.dt.float32, name="tmp", tag="tmp")
                    pre_t = stpool.tile([P, F], mybir.dt.float32, name="pre", tag="pre")
                    nc.scalar.copy(out=tmp_t[:], in_=pslot(j)[:, 0:F])
                    nc.vector.tensor_add(
                        out=pre_t[:],
                        in0=tmp_t[:],
                        in1=pslot(j)[:, F:2 * F],
                    )
                    nc.scalar.activation(
                        out=stage_t[:, j, :],
                        in_=pre_t[:],
                        func=mybir.ActivationFunctionType.Copy,
                        scale=invc_t[:, g_base + j:g_base + j + 1],
                    )
                nc.sync.dma_start(
                    out=out_view[:, g_base:g_base + gs, :],
                    in_=stage_t[:, :gs, :],
                )
                g_base += gs

    nc.compile()
    return nc


def _run(x, edge_idx, trace=False, tmpdir=None):
    from concourse.bass_utils import run_bass_kernel_spmd

    xp, idx_dev, trow_dev, invc_dev, sched, tot, unperm = _host_prep(x, edge_idx)
    nc = _build_program(sched, tot)
    in_maps = [
        {"xp": xp, "idx": idx_dev[k], "trow": trow_dev[k], "invc": invc_dev[k]}
        for k in range(NCORES)
    ]
    kw = dict(trace=True, tmpdir=tmpdir) if trace else {}
    res = run_bass_kernel_spmd(nc, in_maps, list(range(NCORES)), **kw)
    out = np.concatenate(
        [res.results[k]["out"][unperm[k]] for k in range(NCORES)], axis=0
    )
    return out[:N], res.exec_time_ns


def kernel(x, edge_idx):
    return _run(x, edge_idx)[0]

